# revision 15
# baseline (speedup 1.0000x reference)
"""Trainium2 Bass kernel for nn_MessageBuildingLayerLSH.

Strategy (8 NeuronCores, data-parallel over batch B=8, one batch element
per core):
  NEFF1 (binning): per 128-node tile, PE-transpose the x_msg tile, f32
    matmul against [codebook, -codebook] (128x200), then DVE max +
    max_index -> argmax in [0,200) (first-index tie-break, matching
    jnp.argmax). Host adds the mask term and does the cheap stable
    argsort -> bins_split permutation.
  NEFF2 (gather + pairwise): dma_gather rows of x_msg/x_node by the
    permutation (int16 indices), mask-multiply, pairwise L2 distances via
    PE matmul + rank-1 PSUM accumulation of the norms, sqrt/exp on ACT,
    mask outer product via rank-1 matmul, write dm and x_features_binned.
"""

import contextlib
import ctypes
import os
import sys
import types
from contextlib import ExitStack

import ml_dtypes
import numpy as np

import concourse.bacc as bacc
import concourse.bass as bass
import concourse.mybir as mybir
import concourse.tile as tile
from concourse.bass_utils import run_bass_kernel_spmd

# ---------------------------------------------------------------- constants
B, N, DM, DN = 8, 25600, 128, 256
BIN, NB, CB = 128, 200, 100     # bin size, n_bins, codebook slice
NT = N // BIN                   # 200 node tiles
GRP = 4                         # bins per group in NEFF2
NG = NB // GRP

F32 = mybir.dt.float32
BF16 = mybir.dt.bfloat16
I16 = mybir.dt.int16
U32 = mybir.dt.uint32
AF = mybir.ActivationFunctionType
ALU = mybir.AluOpType

# ---------------------------------------------------- walrus wait splitter
# This walrus build rejects >1 sem wait per instruction ("Too many sync
# wait commands"). After Tile scheduling, move extra waits onto nop
# carriers inserted just before the instruction on the same engine
# (engines execute block instructions in order, so semantics match).


def _split_multi_waits(nc):
    for fn in nc.m.functions:
        for bb in fn.blocks:
            new = []
            for inst in bb.instructions:
                si = getattr(inst, "sync_info", None)
                if si is not None and si.on_wait and len(si.on_wait) > 1:
                    waits = list(si.on_wait)
                    for i, w in enumerate(waits[:-1]):
                        new.append(mybir.InstNoOp(
                            name=f"{inst.name}-ws{i}",
                            sync_info=mybir.SyncInfo(on_wait=[w],
                                                     on_update=[]),
                            bass_nofuse=True,
                            engine=inst.engine,
                        ))
                    si.on_wait = [waits[-1]]
                new.append(inst)
            bb.instructions[:] = new

# ------------------------------------------------------- NTFF profile shim
# Recreate the missing ``antenv.axon_hooks`` so trace=True can profile.


def _install_ntff_shim():
    if "antenv.axon_hooks" in sys.modules:
        return
    so_path = "/opt/axon/libaxon_pjrt.so"
    hook = None
    try:
        lib = ctypes.CDLL(so_path)
        if hasattr(lib, "axon_start_nrt_profile"):
            lib.axon_start_nrt_profile.argtypes = [
                ctypes.POINTER(ctypes.c_int64),
                ctypes.c_size_t,
            ]
            lib.axon_start_nrt_profile.restype = ctypes.c_int64
            lib.axon_stop_nrt_profile.argtypes = [ctypes.c_char_p]
            lib.axon_stop_nrt_profile.restype = ctypes.c_int64

            @contextlib.contextmanager
            def _hook(output_dir, device_ids):
                import jax

                jax.devices()
                if device_ids:
                    ids = (ctypes.c_int64 * len(device_ids))(*device_ids)
                    rc = lib.axon_start_nrt_profile(ids, len(device_ids))
                else:
                    rc = lib.axon_start_nrt_profile(None, 0)
                if rc != 0:
                    raise RuntimeError(f"axon_start_nrt_profile rc={rc}")
                try:
                    yield
                finally:
                    n = lib.axon_stop_nrt_profile(str(output_dir).encode())
                    if n <= 0:
                        print(f"ntff profile rc={n} -> {output_dir}",
                              file=sys.stderr)

            hook = _hook
    except OSError:
        pass
    mod = types.ModuleType("antenv.axon_hooks")
    mod.get_axon_ntff_profile_hook = lambda: hook
    mod.set_axon_ntff_profile_hook = lambda h: None
    sys.modules["antenv.axon_hooks"] = mod


_install_ntff_shim()


# ------------------------------------------------------------------- NEFF1
def _build_neff1():
    nc = bacc.Bacc("TRN2", target_bir_lowering=False, debug=False, num_devices=B)
    xmT = nc.dram_tensor("xmT", [DM, N], F32, kind="ExternalInput").ap()
    c2 = nc.dram_tensor("c2", [DM, 2 * CB], F32, kind="ExternalInput").ap()
    amax = nc.dram_tensor("amax", [NT, BIN, 1], U32, kind="ExternalOutput").ap()

    with tile.TileContext(nc) as tc:
        with ExitStack() as ctx:
            consts = ctx.enter_context(tc.tile_pool(name="consts", bufs=1))
            xpool = ctx.enter_context(tc.tile_pool(name="x", bufs=4))
            ppool = ctx.enter_context(tc.tile_pool(name="ps", bufs=3, space="PSUM"))
            mpool = ctx.enter_context(tc.tile_pool(name="mul", bufs=3))
            opool = ctx.enter_context(tc.tile_pool(name="o", bufs=3))

            c2_sb = consts.tile([DM, CB], F32)
            nc.sync.dma_start(c2_sb[:], c2[:, 0:CB])

            for t in range(NT):
                xT_sb = xpool.tile([DM, BIN], F32, tag="xTsb")
                nc.sync.dma_start(xT_sb[:], xmT[:, t * BIN:(t + 1) * BIN])
                mul_ps = ppool.tile([BIN, CB], F32, tag="mul")
                nc.tensor.matmul(mul_ps[:], xT_sb[:], c2_sb[:], start=True,
                                 stop=True)
                mul_sb = mpool.tile([BIN, 2 * CB], F32, tag="mulsb")
                nc.scalar.copy(mul_sb[:, 0:CB], mul_ps[:])
                nc.scalar.mul(mul_sb[:, CB:2 * CB], mul_ps[:], -1.0)
                mx8 = opool.tile([BIN, 8], F32, tag="mx8")
                nc.vector.max(mx8[:], mul_sb[:])
                ix8 = opool.tile([BIN, 8], U32, tag="ix8")
                nc.vector.max_index(ix8[:], mx8[:], mul_sb[:])
                nc.sync.dma_start(amax[t, :, :], ix8[:, 0:1])
    return nc


# ------------------------------------------------------------------- NEFF2
BIG2 = 1.0e5            # 2*BIG mask offset added to d2 of masked rows/cols


def _build_neff2():
    nc = bacc.Bacc("TRN2", target_bir_lowering=False, debug=False, num_devices=B)
    DC = DM + DN           # 320: concatenated [x_msg | x_node] row
    xcat = nc.dram_tensor("xcat", [N, DC], F32, kind="ExternalInput").ap()
    idx = nc.dram_tensor("idx", [128, N // 16], I16, kind="ExternalInput").ap()
    mbc = nc.dram_tensor("mbc", [NB, BIN], F32, kind="ExternalInput").ap()
    mbr = nc.dram_tensor("mbr", [NB, BIN], F32, kind="ExternalInput").ap()
    identb = nc.dram_tensor("identb", [128, 128], BF16,
                            kind="ExternalInput").ap()
    feat = nc.dram_tensor("feat", [N, DN], F32, kind="ExternalOutput").ap()
    dm = nc.dram_tensor("dm", [NB, BIN, BIN], F32, kind="ExternalOutput").ap()

    NIG = GRP * BIN            # idxs per group (512)
    NIC = NIG // 16            # idx cols per group (32)

    with tile.TileContext(nc) as tc:
        with ExitStack() as ctx:
            consts = ctx.enter_context(tc.tile_pool(name="consts", bufs=1))
            gpool = ctx.enter_context(tc.tile_pool(name="g", bufs=3))
            ppool = ctx.enter_context(tc.tile_pool(name="ps", bufs=2,
                                                   space="PSUM"))
            wpool = ctx.enter_context(tc.tile_pool(name="w", bufs=3))
            opool = ctx.enter_context(tc.tile_pool(name="o", bufs=3))

            id_sb = consts.tile([128, 128], BF16)
            nc.sync.dma_start(id_sb[:], identb[:, :])
            idx_sb = consts.tile([128, N // 16], I16)
            nc.sync.dma_start(idx_sb[:], idx[:, :])
            mbc_sb = consts.tile([128, NB], F32)
            nc.sync.dma_start(mbc_sb[:], mbc.rearrange("b n -> n b"))
            mbr_sb = consts.tile([1, N], F32)
            nc.sync.dma_start(mbr_sb[:],
                              mbr.rearrange("b n -> (b n)")[None, :])
            nig_reg = nc.gpsimd.to_reg(NIG)

            feat3 = feat.rearrange("(b n) d -> b n d", n=BIN)

            for g in range(NG):
                b0 = g * GRP
                xc_g = gpool.tile([128, GRP, DC], F32, tag="xc")
                nc.gpsimd.dma_gather(
                    xc_g[:], xcat[:, :], idx_sb[:, g * NIC:(g + 1) * NIC],
                    num_idxs=NIG, num_idxs_reg=nig_reg, elem_size=DC,
                    queue_num=0)
                nc.sync.dma_start(
                    feat3[b0:b0 + GRP].rearrange("b n d -> n b d"),
                    xc_g[:, :, DM:DC])

                # bf16 cast of the x_msg slice
                xb_g = wpool.tile([128, GRP, DM], BF16, tag="xb")
                nc.vector.tensor_copy(xb_g[:], xc_g[:, :, 0:DM])

                # per-bin transpose (PE, bf16 in -> f32 psum -> bf16 sbuf)
                xt_sb = wpool.tile([128, GRP, DM], BF16, tag="xt")
                for k in range(GRP):
                    xt_ps = ppool.tile([DM, BIN], BF16, tag="xtps")
                    nc.tensor.transpose(xt_ps[:], xb_g[:, k, :], id_sb[:])
                    nc.vector.tensor_copy(xt_sb[:, k, :], xt_ps[:])

                # squared norms: row layout for n_i, T layout for n_j
                sqr_g = wpool.tile([128, GRP, DM], F32, tag="sqr")
                nc.scalar.activation(sqr_g[:], xb_g[:], AF.Square)
                ncol_g = wpool.tile([128, GRP], F32, tag="ncol")
                nc.vector.tensor_reduce(ncol_g[:], sqr_g[:],
                                        mybir.AxisListType.X, ALU.add)
                bias_g = wpool.tile([128, GRP], F32, tag="bias")
                nc.vector.tensor_add(bias_g[:], ncol_g[:],
                                     mbc_sb[:, b0:b0 + GRP])

                sqt_g = wpool.tile([128, GRP, DM], F32, tag="sqt")
                nc.scalar.activation(sqt_g[:], xt_sb[:], AF.Square)
                nrow_g = wpool.tile([1, NIG], F32, tag="nrow")
                nc.gpsimd.tensor_reduce(nrow_g[:], sqt_g[:],
                                        mybir.AxisListType.C, ALU.add)
                nsrc_g = wpool.tile([1, NIG], F32, tag="nsrc")
                nc.vector.tensor_add(nsrc_g[:], nrow_g[:],
                                     mbr_sb[0:1, b0 * BIN:(b0 + GRP) * BIN])
                nbc_g = wpool.tile([128, NIG], F32, tag="nbc")
                nc.gpsimd.partition_broadcast(nbc_g[:], nsrc_g[:])

                # pairwise dot products (bf16)
                p_g = ppool.tile([128, GRP, BIN], F32, tag="p")
                for k in range(GRP):
                    nc.tensor.matmul(p_g[:, k, :], xt_sb[:, k, :],
                                     xt_sb[:, k, :], start=True, stop=True)

                # d2 = -2S + n_i + n_j (+ BIG2 on masked rows/cols)
                u_g = opool.tile([128, GRP, BIN], F32, tag="u")
                for k in range(GRP):
                    nc.vector.tensor_scalar(
                        u_g[:, k, :], p_g[:, k, :], -2.0,
                        bias_g[:, k:k + 1], ALU.mult, ALU.add)
                u2_g = opool.tile([128, GRP, BIN], F32, tag="u2")
                nc.vector.tensor_add(
                    u2_g[:], u_g[:],
                    nbc_g.rearrange("p (a b) -> p a b", a=GRP)[:])
                r_g = opool.tile([128, GRP, BIN], F32, tag="r")
                nc.vector.tensor_scalar_max(r_g[:], u2_g[:], 1e-6)
                d_g = opool.tile([128, GRP, BIN], F32, tag="d")
                nc.scalar.activation(d_g[:], r_g[:], AF.Sqrt)
                o_g = opool.tile([128, GRP, BIN], F32, tag="og")
                nc.scalar.activation(o_g[:], d_g[:], AF.Exp, scale=-0.1)
                nc.sync.dma_start(
                    dm[b0:b0 + GRP].rearrange("b i j -> i b j"), o_g[:])
    return nc


# -------------------------------------------------------------------- host
_NC_CACHE = {}
LAST_EXEC_NS = {}


def _get_nc(name, builder):
    if name not in _NC_CACHE:
        nc = builder()
        nc.finalize()
        _split_multi_waits(nc)
        _NC_CACHE[name] = nc
    return _NC_CACHE[name]


def kernel(x_msg, x_node, msk, codebook):
    x_msg = np.ascontiguousarray(x_msg, dtype=np.float32)
    x_node = np.ascontiguousarray(x_node, dtype=np.float32)
    msk = np.ascontiguousarray(msk, dtype=np.float32)
    codebook = np.ascontiguousarray(codebook, dtype=np.float32)
    trace = os.environ.get("KERNEL_PROFILE") == "1"
    ident = np.eye(128, dtype=np.float32)
    c2 = np.concatenate([codebook[:, :CB], -codebook[:, :CB]], axis=1)
    c2 = np.ascontiguousarray(c2, dtype=np.float32)

    # NEFF1: LSH projection + argmax per node
    nc1 = _get_nc("neff1", _build_neff1)
    xmT = np.ascontiguousarray(x_msg.transpose(0, 2, 1))
    in1 = [{"xmT": xmT[b], "c2": c2} for b in range(B)]
    r1 = run_bass_kernel_spmd(nc1, in1, list(range(B)), trace=trace)
    a = np.stack([r1.results[b]["amax"].reshape(N) for b in range(B)])
    a = a.astype(np.int64)

    # host: mask shift + stable argsort (counting-sort-sized problem)
    bin_idx = a + np.where(msk != 0, 0, NB - 1)
    perm = np.argsort(bin_idx, axis=-1, kind="stable")
    bins_split = perm.reshape(B, NB, BIN).astype(np.int32)
    mskb = np.take_along_axis(msk, perm, axis=1).astype(np.float32)
    idx16 = perm.astype(np.int16).reshape(B, N // 16, 16).transpose(0, 2, 1)
    idx16 = np.ascontiguousarray(np.tile(idx16, (1, 8, 1)))  # (B, 128, N/16)

    # NEFF2: gather + pairwise Gaussian kernel
    nc2 = _get_nc("neff2", _build_neff2)
    xcat = np.concatenate([x_msg, x_node], axis=-1)
    mbig = (BIG2 * (1.0 - mskb)).astype(np.float32).reshape(B, NB, BIN)
    identb = np.eye(128, dtype=ml_dtypes.bfloat16)
    in2 = [
        {"xcat": xcat[b], "idx": idx16[b], "mbc": mbig[b], "mbr": mbig[b],
         "identb": identb}
        for b in range(B)
    ]
    r2 = run_bass_kernel_spmd(nc2, in2, list(range(B)), trace=trace)
    feats = np.stack([r2.results[b]["feat"] for b in range(B)])
    feats = feats.reshape(B, NB, BIN, DN)
    dm = np.stack([r2.results[b]["dm"] for b in range(B)])
    dm = dm.reshape(B, NB, BIN, BIN, 1)
    msk_f_binned = mskb.reshape(B, NB, BIN, 1)

    LAST_EXEC_NS.clear()
    LAST_EXEC_NS["neff1"] = r1.exec_time_ns
    LAST_EXEC_NS["neff2"] = r2.exec_time_ns
    LAST_EXEC_NS["r1"] = r1
    LAST_EXEC_NS["r2"] = r2
    return bins_split, feats, dm, msk_f_binned


# revision 20
# speedup vs baseline: 3.2791x; 3.2791x over previous
"""Trainium2 Bass kernel for nn_MessageBuildingLayerLSH.

Strategy (8 NeuronCores, data-parallel over batch B=8, one batch element
per core):
  NEFF1 (binning): per 128-node tile, PE-transpose the x_msg tile, f32
    matmul against [codebook, -codebook] (128x200), then DVE max +
    max_index -> argmax in [0,200) (first-index tie-break, matching
    jnp.argmax). Host adds the mask term and does the cheap stable
    argsort -> bins_split permutation.
  NEFF2 (gather + pairwise): dma_gather rows of x_msg/x_node by the
    permutation (int16 indices), mask-multiply, pairwise L2 distances via
    PE matmul + rank-1 PSUM accumulation of the norms, sqrt/exp on ACT,
    mask outer product via rank-1 matmul, write dm and x_features_binned.
"""

import contextlib
import ctypes
import os
import sys
import types
from contextlib import ExitStack

import ml_dtypes
import numpy as np

import concourse.bacc as bacc
import concourse.bass as bass
import concourse.mybir as mybir
import concourse.tile as tile
from concourse.bass_utils import run_bass_kernel_spmd

# ---------------------------------------------------------------- constants
B, N, DM, DN = 8, 25600, 128, 256
BIN, NB, CB = 128, 200, 100     # bin size, n_bins, codebook slice
NT = N // BIN                   # 200 node tiles
GRP = 4                         # bins per group in NEFF2
NG = NB // GRP

F32 = mybir.dt.float32
BF16 = mybir.dt.bfloat16
I16 = mybir.dt.int16
U32 = mybir.dt.uint32
AF = mybir.ActivationFunctionType
ALU = mybir.AluOpType

# ---------------------------------------------------- walrus wait splitter
# This walrus build rejects >1 sem wait per instruction ("Too many sync
# wait commands"). After Tile scheduling, move extra waits onto nop
# carriers inserted just before the instruction on the same engine
# (engines execute block instructions in order, so semantics match).


def _split_multi_waits(nc):
    for fn in nc.m.functions:
        for bb in fn.blocks:
            new = []
            for inst in bb.instructions:
                si = getattr(inst, "sync_info", None)
                if si is not None and si.on_wait and len(si.on_wait) > 1:
                    waits = list(si.on_wait)
                    for i, w in enumerate(waits[:-1]):
                        new.append(mybir.InstNoOp(
                            name=f"{inst.name}-ws{i}",
                            sync_info=mybir.SyncInfo(on_wait=[w],
                                                     on_update=[]),
                            bass_nofuse=True,
                            engine=inst.engine,
                        ))
                    si.on_wait = [waits[-1]]
                new.append(inst)
            bb.instructions[:] = new

# ------------------------------------------------------- NTFF profile shim
# Recreate the missing ``antenv.axon_hooks`` so trace=True can profile.


def _install_ntff_shim():
    if "antenv.axon_hooks" in sys.modules:
        return
    so_path = "/opt/axon/libaxon_pjrt.so"
    hook = None
    try:
        lib = ctypes.CDLL(so_path)
        if hasattr(lib, "axon_start_nrt_profile"):
            lib.axon_start_nrt_profile.argtypes = [
                ctypes.POINTER(ctypes.c_int64),
                ctypes.c_size_t,
            ]
            lib.axon_start_nrt_profile.restype = ctypes.c_int64
            lib.axon_stop_nrt_profile.argtypes = [ctypes.c_char_p]
            lib.axon_stop_nrt_profile.restype = ctypes.c_int64

            @contextlib.contextmanager
            def _hook(output_dir, device_ids):
                import jax

                jax.devices()
                if device_ids:
                    ids = (ctypes.c_int64 * len(device_ids))(*device_ids)
                    rc = lib.axon_start_nrt_profile(ids, len(device_ids))
                else:
                    rc = lib.axon_start_nrt_profile(None, 0)
                if rc != 0:
                    raise RuntimeError(f"axon_start_nrt_profile rc={rc}")
                try:
                    yield
                finally:
                    n = lib.axon_stop_nrt_profile(str(output_dir).encode())
                    if n <= 0:
                        print(f"ntff profile rc={n} -> {output_dir}",
                              file=sys.stderr)

            hook = _hook
    except OSError:
        pass
    mod = types.ModuleType("antenv.axon_hooks")
    mod.get_axon_ntff_profile_hook = lambda: hook
    mod.set_axon_ntff_profile_hook = lambda h: None
    sys.modules["antenv.axon_hooks"] = mod


_install_ntff_shim()


# ------------------------------------------------------------------- NEFF1
def _build_neff1():
    nc = bacc.Bacc("TRN2", target_bir_lowering=False, debug=False, num_devices=B)
    xmT = nc.dram_tensor("xmT", [DM, N], F32, kind="ExternalInput").ap()
    c2 = nc.dram_tensor("c2", [DM, 2 * CB], F32, kind="ExternalInput").ap()
    amax = nc.dram_tensor("amax", [NT, BIN, 1], U32, kind="ExternalOutput").ap()

    with tile.TileContext(nc) as tc:
        with ExitStack() as ctx:
            consts = ctx.enter_context(tc.tile_pool(name="consts", bufs=1))
            xpool = ctx.enter_context(tc.tile_pool(name="x", bufs=4))
            ppool = ctx.enter_context(tc.tile_pool(name="ps", bufs=3, space="PSUM"))
            mpool = ctx.enter_context(tc.tile_pool(name="mul", bufs=3))
            opool = ctx.enter_context(tc.tile_pool(name="o", bufs=3))

            c2_sb = consts.tile([DM, CB], F32)
            nc.sync.dma_start(c2_sb[:], c2[:, 0:CB])

            for t in range(NT):
                xT_sb = xpool.tile([DM, BIN], F32, tag="xTsb")
                nc.sync.dma_start(xT_sb[:], xmT[:, t * BIN:(t + 1) * BIN])
                mul_ps = ppool.tile([BIN, CB], F32, tag="mul")
                nc.tensor.matmul(mul_ps[:], xT_sb[:], c2_sb[:], start=True,
                                 stop=True)
                mul_sb = mpool.tile([BIN, 2 * CB], F32, tag="mulsb")
                nc.scalar.copy(mul_sb[:, 0:CB], mul_ps[:])
                nc.scalar.mul(mul_sb[:, CB:2 * CB], mul_ps[:], -1.0)
                mx8 = opool.tile([BIN, 8], F32, tag="mx8")
                nc.vector.max(mx8[:], mul_sb[:])
                ix8 = opool.tile([BIN, 8], U32, tag="ix8")
                nc.vector.max_index(ix8[:], mx8[:], mul_sb[:])
                nc.sync.dma_start(amax[t, :, :], ix8[:, 0:1])
    return nc


# ------------------------------------------------------------------- NEFF2
BIG2 = 1.0e5            # 2*BIG mask offset added to d2 of masked rows/cols


def _build_neff2():
    nc = bacc.Bacc("TRN2", target_bir_lowering=False, debug=False, num_devices=B)
    DC = DM + DN           # 320: concatenated [x_msg | x_node] row
    xcat = nc.dram_tensor("xcat", [N, DC], F32, kind="ExternalInput").ap()
    idx = nc.dram_tensor("idx", [128, N // 16], I16, kind="ExternalInput").ap()
    mbc = nc.dram_tensor("mbc", [NB, BIN], F32, kind="ExternalInput").ap()
    mbr = nc.dram_tensor("mbr", [NB, BIN], F32, kind="ExternalInput").ap()
    identb = nc.dram_tensor("identb", [128, 128], BF16,
                            kind="ExternalInput").ap()
    feat = nc.dram_tensor("feat", [N, DN], F32, kind="ExternalOutput").ap()
    dm = nc.dram_tensor("dm", [NB, BIN, BIN], F32, kind="ExternalOutput").ap()
    nscr = nc.dram_tensor("nscr", [NB // GRP, GRP, BIN], F32).ap()

    NIG = GRP * BIN            # idxs per group (512)
    NIC = NIG // 16            # idx cols per group (32)

    with tile.TileContext(nc) as tc:
        with ExitStack() as ctx:
            consts = ctx.enter_context(tc.tile_pool(name="consts", bufs=1))
            gpool = ctx.enter_context(tc.tile_pool(name="g", bufs=3))
            ppool = ctx.enter_context(tc.tile_pool(name="ps", bufs=2,
                                                   space="PSUM"))
            wpool = ctx.enter_context(tc.tile_pool(name="w", bufs=3))
            opool = ctx.enter_context(tc.tile_pool(name="o", bufs=3))

            id_sb = consts.tile([128, 128], BF16)
            nc.sync.dma_start(id_sb[:], identb[:, :])
            idx_sb = consts.tile([128, N // 16], I16)
            nc.sync.dma_start(idx_sb[:], idx[:, :])
            mbc_sb = consts.tile([128, NB], F32)
            nc.sync.dma_start(mbc_sb[:], mbc.rearrange("b n -> n b"))
            mbr_sb = consts.tile([1, N], F32)
            nc.sync.dma_start(mbr_sb[:],
                              mbr.rearrange("b n -> (b n)")[None, :])
            ones_sb = consts.tile([1, BIN], BF16)
            nc.vector.memset(ones_sb[:], 1.0)
            nig_reg = nc.gpsimd.to_reg(NIG)

            feat3 = feat.rearrange("(b n) d -> b n d", n=BIN)

            for g in range(NG):
                b0 = g * GRP
                xc_g = gpool.tile([128, GRP, DC], F32, tag="xc")
                nc.gpsimd.dma_gather(
                    xc_g[:], xcat[:, :], idx_sb[:, g * NIC:(g + 1) * NIC],
                    num_idxs=NIG, num_idxs_reg=nig_reg, elem_size=DC,
                    queue_num=0)
                nc.sync.dma_start(
                    feat3[b0:b0 + GRP].rearrange("b n d -> n b d"),
                    xc_g[:, :, DM:DC])

                # bf16 cast of the x_msg slice
                xb_g = wpool.tile([128, GRP, DM], BF16, tag="xb")
                nc.vector.tensor_copy(xb_g[:], xc_g[:, :, 0:DM])

                # per-bin transpose (PE, bf16 in -> f32 psum -> bf16 sbuf)
                xt_sb = wpool.tile([128, GRP, DM], BF16, tag="xt")
                for k in range(GRP):
                    xt_ps = ppool.tile([DM, BIN], BF16, tag="xtps")
                    nc.tensor.transpose(xt_ps[:], xb_g[:, k, :], id_sb[:])
                    nc.vector.tensor_copy(xt_sb[:, k, :], xt_ps[:])

                # squared norms (row layout): n_col for the n_i bias
                sqr_g = wpool.tile([128, GRP, DM], F32, tag="sqr")
                nc.scalar.activation(sqr_g[:], xb_g[:], AF.Square)
                ncol_g = wpool.tile([128, GRP], F32, tag="ncol")
                nc.vector.tensor_reduce(ncol_g[:], sqr_g[:],
                                        mybir.AxisListType.X, ALU.add)
                bias_g = wpool.tile([128, GRP], F32, tag="bias")
                nc.vector.tensor_add(bias_g[:], ncol_g[:],
                                     mbc_sb[:, b0:b0 + GRP])

                # n as a row: DMA round-trip (column -> DRAM -> row)
                nc.sync.dma_start(nscr[g].rearrange("a p -> p a"), ncol_g[:])
                nrow_g = wpool.tile([1, NIG], F32, tag="nrow")
                nc.sync.dma_start(nrow_g[:],
                                  nscr[g].rearrange("a p -> (a p)")[None, :])
                # nis = -0.5*n_j - 0.5*BIG2*(1-m_j), split hi/lo bf16
                nis_g = wpool.tile([1, NIG], F32, tag="nis")
                nc.vector.tensor_scalar(nis_g[:], nrow_g[:], -0.5,
                                        None, ALU.mult)
                nc.vector.tensor_add(nis_g[:], nis_g[:],
                                     mbr_sb[0:1, b0 * BIN:(b0 + GRP) * BIN])
                nhi_g = wpool.tile([1, NIG], BF16, tag="nhi")
                nc.vector.tensor_copy(nhi_g[:], nis_g[:])
                nhf_g = wpool.tile([1, NIG], F32, tag="nhf")
                nc.vector.tensor_copy(nhf_g[:], nhi_g[:])
                nlo_g = wpool.tile([1, NIG], BF16, tag="nlo")
                nc.vector.tensor_sub(nlo_g[:], nis_g[:], nhf_g[:])

                # P = ones (x) n_j (hi+lo) + per-bin X.X^T, one PSUM group
                p_g = ppool.tile([128, GRP, BIN], F32, tag="p")
                pflat = p_g.rearrange("p a b -> p (a b)")
                nc.tensor.matmul(pflat[:], ones_sb[:], nhi_g[:],
                                 start=True, stop=False)
                nc.tensor.matmul(pflat[:], ones_sb[:], nlo_g[:],
                                 start=False, stop=False)
                for k in range(GRP):
                    nc.tensor.matmul(p_g[:, k, :], xt_sb[:, k, :],
                                     xt_sb[:, k, :], start=False,
                                     stop=(k == GRP - 1))

                # d2 = -2P + n_i (+ BIG2 on masked rows)
                u_g = opool.tile([128, GRP, BIN], F32, tag="u")
                for k in range(GRP):
                    nc.vector.tensor_scalar(
                        u_g[:, k, :], p_g[:, k, :], -2.0,
                        bias_g[:, k:k + 1], ALU.mult, ALU.add)
                r_g = opool.tile([128, GRP, BIN], F32, tag="r")
                nc.vector.tensor_scalar_max(r_g[:], u_g[:], 1e-6)
                d_g = opool.tile([128, GRP, BIN], F32, tag="d")
                nc.scalar.activation(d_g[:], r_g[:], AF.Sqrt)
                o_g = opool.tile([128, GRP, BIN], F32, tag="og")
                nc.scalar.activation(o_g[:], d_g[:], AF.Exp, scale=-0.1)
                nc.sync.dma_start(
                    dm[b0:b0 + GRP].rearrange("b i j -> i b j"), o_g[:])
    return nc


# -------------------------------------------------------------------- host
_NC_CACHE = {}
LAST_EXEC_NS = {}


def _get_nc(name, builder):
    if name not in _NC_CACHE:
        nc = builder()
        nc.finalize()
        _split_multi_waits(nc)
        _NC_CACHE[name] = nc
    return _NC_CACHE[name]


def kernel(x_msg, x_node, msk, codebook):
    x_msg = np.ascontiguousarray(x_msg, dtype=np.float32)
    x_node = np.ascontiguousarray(x_node, dtype=np.float32)
    msk = np.ascontiguousarray(msk, dtype=np.float32)
    codebook = np.ascontiguousarray(codebook, dtype=np.float32)
    trace = os.environ.get("KERNEL_PROFILE") == "1"
    ident = np.eye(128, dtype=np.float32)
    c2 = np.concatenate([codebook[:, :CB], -codebook[:, :CB]], axis=1)
    c2 = np.ascontiguousarray(c2, dtype=np.float32)

    # NEFF1: LSH projection + argmax per node
    nc1 = _get_nc("neff1", _build_neff1)
    xmT = np.ascontiguousarray(x_msg.transpose(0, 2, 1))
    in1 = [{"xmT": xmT[b], "c2": c2} for b in range(B)]
    r1 = run_bass_kernel_spmd(nc1, in1, list(range(B)), trace=trace)
    a = np.stack([r1.results[b]["amax"].reshape(N) for b in range(B)])
    a = a.astype(np.int64)

    # host: mask shift + stable argsort (counting-sort-sized problem)
    bin_idx = a + np.where(msk != 0, 0, NB - 1)
    perm = np.argsort(bin_idx, axis=-1, kind="stable")
    bins_split = perm.reshape(B, NB, BIN).astype(np.int32)
    mskb = np.take_along_axis(msk, perm, axis=1).astype(np.float32)
    idx16 = perm.astype(np.int16).reshape(B, N // 16, 16).transpose(0, 2, 1)
    idx16 = np.ascontiguousarray(np.tile(idx16, (1, 8, 1)))  # (B, 128, N/16)

    # NEFF2: gather + pairwise Gaussian kernel
    nc2 = _get_nc("neff2", _build_neff2)
    xcat = np.concatenate([x_msg, x_node], axis=-1)
    mbig = (BIG2 * (1.0 - mskb)).astype(np.float32).reshape(B, NB, BIN)
    mbrh = (-0.5 * mbig).astype(np.float32)
    identb = np.eye(128, dtype=ml_dtypes.bfloat16)
    in2 = [
        {"xcat": xcat[b], "idx": idx16[b], "mbc": mbig[b], "mbr": mbrh[b],
         "identb": identb}
        for b in range(B)
    ]
    r2 = run_bass_kernel_spmd(nc2, in2, list(range(B)), trace=trace)
    feats = np.stack([r2.results[b]["feat"] for b in range(B)])
    feats = feats.reshape(B, NB, BIN, DN)
    dm = np.stack([r2.results[b]["dm"] for b in range(B)])
    dm = dm.reshape(B, NB, BIN, BIN, 1)
    msk_f_binned = mskb.reshape(B, NB, BIN, 1)

    LAST_EXEC_NS.clear()
    LAST_EXEC_NS["neff1"] = r1.exec_time_ns
    LAST_EXEC_NS["neff2"] = r2.exec_time_ns
    LAST_EXEC_NS["r1"] = r1
    LAST_EXEC_NS["r2"] = r2
    return bins_split, feats, dm, msk_f_binned


# revision 22
# speedup vs baseline: 3.4446x; 1.0505x over previous
"""Trainium2 Bass kernel for nn_MessageBuildingLayerLSH.

Strategy (8 NeuronCores, data-parallel over batch B=8, one batch element
per core):
  NEFF1 (binning): per 128-node tile, PE-transpose the x_msg tile, f32
    matmul against [codebook, -codebook] (128x200), then DVE max +
    max_index -> argmax in [0,200) (first-index tie-break, matching
    jnp.argmax). Host adds the mask term and does the cheap stable
    argsort -> bins_split permutation.
  NEFF2 (gather + pairwise): dma_gather rows of x_msg/x_node by the
    permutation (int16 indices), mask-multiply, pairwise L2 distances via
    PE matmul + rank-1 PSUM accumulation of the norms, sqrt/exp on ACT,
    mask outer product via rank-1 matmul, write dm and x_features_binned.
"""

import contextlib
import ctypes
import os
import sys
import types
from contextlib import ExitStack

import ml_dtypes
import numpy as np

import concourse.bacc as bacc
import concourse.bass as bass
import concourse.mybir as mybir
import concourse.tile as tile
from concourse.bass_utils import run_bass_kernel_spmd

# ---------------------------------------------------------------- constants
B, N, DM, DN = 8, 25600, 128, 256
BIN, NB, CB = 128, 200, 100     # bin size, n_bins, codebook slice
NT = N // BIN                   # 200 node tiles
GRP = 4                         # bins per group in NEFF2
NG = NB // GRP

F32 = mybir.dt.float32
BF16 = mybir.dt.bfloat16
I16 = mybir.dt.int16
U32 = mybir.dt.uint32
AF = mybir.ActivationFunctionType
ALU = mybir.AluOpType

# ---------------------------------------------------- walrus wait splitter
# This walrus build rejects >1 sem wait per instruction ("Too many sync
# wait commands"). After Tile scheduling, move extra waits onto nop
# carriers inserted just before the instruction on the same engine
# (engines execute block instructions in order, so semantics match).


def _split_multi_waits(nc):
    for fn in nc.m.functions:
        for bb in fn.blocks:
            new = []
            for inst in bb.instructions:
                si = getattr(inst, "sync_info", None)
                if si is not None and si.on_wait and len(si.on_wait) > 1:
                    waits = list(si.on_wait)
                    for i, w in enumerate(waits[:-1]):
                        new.append(mybir.InstNoOp(
                            name=f"{inst.name}-ws{i}",
                            sync_info=mybir.SyncInfo(on_wait=[w],
                                                     on_update=[]),
                            bass_nofuse=True,
                            engine=inst.engine,
                        ))
                    si.on_wait = [waits[-1]]
                new.append(inst)
            bb.instructions[:] = new

# ------------------------------------------------------- NTFF profile shim
# Recreate the missing ``antenv.axon_hooks`` so trace=True can profile.


def _install_ntff_shim():
    if "antenv.axon_hooks" in sys.modules:
        return
    so_path = "/opt/axon/libaxon_pjrt.so"
    hook = None
    try:
        lib = ctypes.CDLL(so_path)
        if hasattr(lib, "axon_start_nrt_profile"):
            lib.axon_start_nrt_profile.argtypes = [
                ctypes.POINTER(ctypes.c_int64),
                ctypes.c_size_t,
            ]
            lib.axon_start_nrt_profile.restype = ctypes.c_int64
            lib.axon_stop_nrt_profile.argtypes = [ctypes.c_char_p]
            lib.axon_stop_nrt_profile.restype = ctypes.c_int64

            @contextlib.contextmanager
            def _hook(output_dir, device_ids):
                import jax

                jax.devices()
                if device_ids:
                    ids = (ctypes.c_int64 * len(device_ids))(*device_ids)
                    rc = lib.axon_start_nrt_profile(ids, len(device_ids))
                else:
                    rc = lib.axon_start_nrt_profile(None, 0)
                if rc != 0:
                    raise RuntimeError(f"axon_start_nrt_profile rc={rc}")
                try:
                    yield
                finally:
                    n = lib.axon_stop_nrt_profile(str(output_dir).encode())
                    if n <= 0:
                        print(f"ntff profile rc={n} -> {output_dir}",
                              file=sys.stderr)

            hook = _hook
    except OSError:
        pass
    mod = types.ModuleType("antenv.axon_hooks")
    mod.get_axon_ntff_profile_hook = lambda: hook
    mod.set_axon_ntff_profile_hook = lambda h: None
    sys.modules["antenv.axon_hooks"] = mod


_install_ntff_shim()


# ------------------------------------------------------------------- NEFF1
def _build_neff1():
    nc = bacc.Bacc("TRN2", target_bir_lowering=False, debug=False, num_devices=B)
    xmT = nc.dram_tensor("xmT", [DM, N], F32, kind="ExternalInput").ap()
    c2 = nc.dram_tensor("c2", [DM, 2 * CB], F32, kind="ExternalInput").ap()
    amax = nc.dram_tensor("amax", [NT, BIN, 1], U32, kind="ExternalOutput").ap()

    with tile.TileContext(nc) as tc:
        with ExitStack() as ctx:
            consts = ctx.enter_context(tc.tile_pool(name="consts", bufs=1))
            xpool = ctx.enter_context(tc.tile_pool(name="x", bufs=6))
            ppool = ctx.enter_context(tc.tile_pool(name="ps", bufs=4, space="PSUM"))
            mpool = ctx.enter_context(tc.tile_pool(name="mul", bufs=4))
            opool = ctx.enter_context(tc.tile_pool(name="o", bufs=3))

            c2_sb = consts.tile([DM, CB], F32)
            nc.sync.dma_start(c2_sb[:], c2[:, 0:CB])

            for t in range(NT):
                xT_sb = xpool.tile([DM, BIN], F32, tag="xTsb")
                nc.sync.dma_start(xT_sb[:], xmT[:, t * BIN:(t + 1) * BIN])
                mul_ps = ppool.tile([BIN, CB], F32, tag="mul")
                nc.tensor.matmul(mul_ps[:], xT_sb[:], c2_sb[:], start=True,
                                 stop=True)
                mul_sb = mpool.tile([BIN, 2 * CB], F32, tag="mulsb")
                nc.scalar.copy(mul_sb[:, 0:CB], mul_ps[:])
                nc.scalar.mul(mul_sb[:, CB:2 * CB], mul_ps[:], -1.0)
                mx8 = opool.tile([BIN, 8], F32, tag="mx8")
                nc.vector.max(mx8[:], mul_sb[:])
                ix8 = opool.tile([BIN, 8], U32, tag="ix8")
                nc.vector.max_index(ix8[:], mx8[:], mul_sb[:])
                nc.sync.dma_start(amax[t, :, :], ix8[:, 0:1])
    return nc


# ------------------------------------------------------------------- NEFF2
BIG2 = 1.0e5            # 2*BIG mask offset added to d2 of masked rows/cols


def _build_neff2():
    nc = bacc.Bacc("TRN2", target_bir_lowering=False, debug=False, num_devices=B)
    DC = DM + DN           # 320: concatenated [x_msg | x_node] row
    xcat = nc.dram_tensor("xcat", [N, DC], F32, kind="ExternalInput").ap()
    idx = nc.dram_tensor("idx", [128, N // 16], I16, kind="ExternalInput").ap()
    mbc = nc.dram_tensor("mbc", [NB, BIN], F32, kind="ExternalInput").ap()
    mbr = nc.dram_tensor("mbr", [NB, BIN], F32, kind="ExternalInput").ap()
    identb = nc.dram_tensor("identb", [128, 128], BF16,
                            kind="ExternalInput").ap()
    feat = nc.dram_tensor("feat", [N, DN], F32, kind="ExternalOutput").ap()
    dm = nc.dram_tensor("dm", [NB, BIN, BIN], F32, kind="ExternalOutput").ap()
    nscr = nc.dram_tensor("nscr", [NB // GRP, GRP, BIN], F32).ap()

    NIG = GRP * BIN            # idxs per group (512)
    NIC = NIG // 16            # idx cols per group (32)

    with tile.TileContext(nc) as tc:
        with ExitStack() as ctx:
            consts = ctx.enter_context(tc.tile_pool(name="consts", bufs=1))
            gpool = ctx.enter_context(tc.tile_pool(name="g", bufs=4))
            ppool = ctx.enter_context(tc.tile_pool(name="ps", bufs=3,
                                                   space="PSUM"))
            wpool = ctx.enter_context(tc.tile_pool(name="w", bufs=4))
            opool = ctx.enter_context(tc.tile_pool(name="o", bufs=3))

            id_sb = consts.tile([128, 128], BF16)
            nc.sync.dma_start(id_sb[:], identb[:, :])
            idx_sb = consts.tile([128, N // 16], I16)
            nc.sync.dma_start(idx_sb[:], idx[:, :])
            mbc_sb = consts.tile([128, NB], F32)
            nc.sync.dma_start(mbc_sb[:], mbc.rearrange("b n -> n b"))
            mbr_sb = consts.tile([1, N], F32)
            nc.sync.dma_start(mbr_sb[:],
                              mbr.rearrange("b n -> (b n)")[None, :])
            ones_sb = consts.tile([1, BIN], BF16)
            nc.vector.memset(ones_sb[:], 1.0)
            nig_reg = nc.gpsimd.to_reg(NIG)

            feat3 = feat.rearrange("(b n) d -> b n d", n=BIN)

            for g in range(NG):
                b0 = g * GRP
                xc_g = gpool.tile([128, GRP, DC], F32, tag="xc")
                nc.gpsimd.dma_gather(
                    xc_g[:], xcat[:, :], idx_sb[:, g * NIC:(g + 1) * NIC],
                    num_idxs=NIG, num_idxs_reg=nig_reg, elem_size=DC,
                    queue_num=0)
                nc.sync.dma_start(
                    feat3[b0:b0 + GRP].rearrange("b n d -> n b d"),
                    xc_g[:, :, DM:DC])

                # bf16 cast of the x_msg slice
                xb_g = wpool.tile([128, GRP, DM], BF16, tag="xb")
                nc.vector.tensor_copy(xb_g[:], xc_g[:, :, 0:DM])

                # per-bin transpose (PE, bf16 in -> f32 psum -> bf16 sbuf)
                xt_sb = wpool.tile([128, GRP, DM], BF16, tag="xt")
                for k in range(GRP):
                    xt_ps = ppool.tile([DM, BIN], BF16, tag="xtps")
                    nc.tensor.transpose(xt_ps[:], xb_g[:, k, :], id_sb[:])
                    nc.vector.tensor_copy(xt_sb[:, k, :], xt_ps[:])

                # squared norms (row layout): n_col for the n_i bias
                sqr_g = wpool.tile([128, GRP, DM], F32, tag="sqr")
                nc.vector.tensor_mul(sqr_g[:], xb_g[:], xb_g[:])
                ncol_g = wpool.tile([128, GRP], F32, tag="ncol")
                nc.vector.tensor_reduce(ncol_g[:], sqr_g[:],
                                        mybir.AxisListType.X, ALU.add)
                bias_g = wpool.tile([128, GRP], F32, tag="bias")
                nc.vector.tensor_add(bias_g[:], ncol_g[:],
                                     mbc_sb[:, b0:b0 + GRP])

                # n as a row: DMA round-trip (column -> DRAM -> row)
                nc.sync.dma_start(nscr[g].rearrange("a p -> p a"), ncol_g[:])
                nrow_g = wpool.tile([1, NIG], F32, tag="nrow")
                nc.sync.dma_start(nrow_g[:],
                                  nscr[g].rearrange("a p -> (a p)")[None, :])
                # nis = -0.5*n_j - 0.5*BIG2*(1-m_j), split hi/lo bf16
                nis_g = wpool.tile([1, NIG], F32, tag="nis")
                nc.vector.tensor_scalar(nis_g[:], nrow_g[:], -0.5,
                                        None, ALU.mult)
                nc.vector.tensor_add(nis_g[:], nis_g[:],
                                     mbr_sb[0:1, b0 * BIN:(b0 + GRP) * BIN])
                nhi_g = wpool.tile([1, NIG], BF16, tag="nhi")
                nc.vector.tensor_copy(nhi_g[:], nis_g[:])
                nhf_g = wpool.tile([1, NIG], F32, tag="nhf")
                nc.vector.tensor_copy(nhf_g[:], nhi_g[:])
                nlo_g = wpool.tile([1, NIG], BF16, tag="nlo")
                nc.vector.tensor_sub(nlo_g[:], nis_g[:], nhf_g[:])

                # P = ones (x) n_j (hi+lo) + per-bin X.X^T, one PSUM group
                p_g = ppool.tile([128, GRP, BIN], F32, tag="p")
                pflat = p_g.rearrange("p a b -> p (a b)")
                nc.tensor.matmul(pflat[:], ones_sb[:], nhi_g[:],
                                 start=True, stop=False)
                nc.tensor.matmul(pflat[:], ones_sb[:], nlo_g[:],
                                 start=False, stop=False)
                for k in range(GRP):
                    nc.tensor.matmul(p_g[:, k, :], xt_sb[:, k, :],
                                     xt_sb[:, k, :], start=False,
                                     stop=(k == GRP - 1))

                # d2 = -2P + n_i (+ BIG2 on masked rows)
                u_g = opool.tile([128, GRP, BIN], F32, tag="u")
                for k in range(GRP):
                    nc.vector.tensor_scalar(
                        u_g[:, k, :], p_g[:, k, :], -2.0,
                        bias_g[:, k:k + 1], ALU.mult, ALU.add)
                r_g = opool.tile([128, GRP, BIN], F32, tag="r")
                nc.vector.tensor_scalar_max(r_g[:], u_g[:], 1e-6)
                d_g = opool.tile([128, GRP, BIN], F32, tag="d")
                nc.scalar.activation(d_g[:], r_g[:], AF.Sqrt)
                o_g = opool.tile([128, GRP, BIN], F32, tag="og")
                nc.scalar.activation(o_g[:], d_g[:], AF.Exp, scale=-0.1)
                nc.sync.dma_start(
                    dm[b0:b0 + GRP].rearrange("b i j -> i b j"), o_g[:])
    return nc


# -------------------------------------------------------------------- host
_NC_CACHE = {}
LAST_EXEC_NS = {}


def _get_nc(name, builder):
    if name not in _NC_CACHE:
        nc = builder()
        nc.finalize()
        _split_multi_waits(nc)
        _NC_CACHE[name] = nc
    return _NC_CACHE[name]


def kernel(x_msg, x_node, msk, codebook):
    x_msg = np.ascontiguousarray(x_msg, dtype=np.float32)
    x_node = np.ascontiguousarray(x_node, dtype=np.float32)
    msk = np.ascontiguousarray(msk, dtype=np.float32)
    codebook = np.ascontiguousarray(codebook, dtype=np.float32)
    trace = os.environ.get("KERNEL_PROFILE") == "1"
    ident = np.eye(128, dtype=np.float32)
    c2 = np.concatenate([codebook[:, :CB], -codebook[:, :CB]], axis=1)
    c2 = np.ascontiguousarray(c2, dtype=np.float32)

    # NEFF1: LSH projection + argmax per node
    nc1 = _get_nc("neff1", _build_neff1)
    xmT = np.ascontiguousarray(x_msg.transpose(0, 2, 1))
    in1 = [{"xmT": xmT[b], "c2": c2} for b in range(B)]
    r1 = run_bass_kernel_spmd(nc1, in1, list(range(B)), trace=trace)
    a = np.stack([r1.results[b]["amax"].reshape(N) for b in range(B)])
    a = a.astype(np.int64)

    # host: mask shift + stable argsort (counting-sort-sized problem)
    bin_idx = a + np.where(msk != 0, 0, NB - 1)
    perm = np.argsort(bin_idx, axis=-1, kind="stable")
    bins_split = perm.reshape(B, NB, BIN).astype(np.int32)
    mskb = np.take_along_axis(msk, perm, axis=1).astype(np.float32)
    idx16 = perm.astype(np.int16).reshape(B, N // 16, 16).transpose(0, 2, 1)
    idx16 = np.ascontiguousarray(np.tile(idx16, (1, 8, 1)))  # (B, 128, N/16)

    # NEFF2: gather + pairwise Gaussian kernel
    nc2 = _get_nc("neff2", _build_neff2)
    xcat = np.concatenate([x_msg, x_node], axis=-1)
    mbig = (BIG2 * (1.0 - mskb)).astype(np.float32).reshape(B, NB, BIN)
    mbrh = (-0.5 * mbig).astype(np.float32)
    identb = np.eye(128, dtype=ml_dtypes.bfloat16)
    in2 = [
        {"xcat": xcat[b], "idx": idx16[b], "mbc": mbig[b], "mbr": mbrh[b],
         "identb": identb}
        for b in range(B)
    ]
    r2 = run_bass_kernel_spmd(nc2, in2, list(range(B)), trace=trace)
    feats = np.stack([r2.results[b]["feat"] for b in range(B)])
    feats = feats.reshape(B, NB, BIN, DN)
    dm = np.stack([r2.results[b]["dm"] for b in range(B)])
    dm = dm.reshape(B, NB, BIN, BIN, 1)
    msk_f_binned = mskb.reshape(B, NB, BIN, 1)

    LAST_EXEC_NS.clear()
    LAST_EXEC_NS["neff1"] = r1.exec_time_ns
    LAST_EXEC_NS["neff2"] = r2.exec_time_ns
    LAST_EXEC_NS["r1"] = r1
    LAST_EXEC_NS["r2"] = r2
    return bins_split, feats, dm, msk_f_binned
